# revision 7
# baseline (speedup 1.0000x reference)
"""Trainium2 Bass kernel for GNN mean aggregation (nn_AggrGSMean).

Computes, for t in {0,1}:
    out_t[b, v, :] = segment_sum(features_t over edges with dest v) / degree[b, v, t]
where degree[b, v, t] = max(count(adjacency[b, v, t, :] >= 0), 1).

Strategy (graph-partition sharding per the problem's sharding hint):
- Host: partition edges by destination-vertex range across 8 cores, sort each
  core's edges by destination, group into 128-vertex blocks.  Each block's edge
  list is padded to a whole number of 128-edge tiles.  Blocks are assigned to
  "slots" in decreasing-tile-count order so one static per-slot tile profile
  (max over cores/tables at each rank) serves all cores with ~8% less padding
  than a uniform max.  Features ship as bf16 hi+lo halves (their sum is the
  fp32 value to ~1e-5) plus the destination slot-in-block encoded as a float.
- Device (per core): for each slot, stream 128-edge tiles [hi64|lo64|negv]
  bf16; build a one-hot [128 edges x 128 vslots] in bf16 (iota == vslot) on
  DVE (a fraction on ScalarE via relu(1-(iota-v)^2)); one matmul per tile
  accumulates onehot.T @ [hi|lo] into PSUM [128, 128].  Degree comes from the
  adjacency slice on-chip; the hi/lo halves are summed by a strided
  tensor_reduce and the mean division rides the ScalarE copy (per-partition
  scale = 1/degree).
"""

import sys

if "/opt/trn_rl_repo" not in sys.path:
    sys.path.insert(0, "/opt/trn_rl_repo")

import ml_dtypes
import numpy as np

# Problem constants (hardcoded per contract)
B, V, T, N, F, M = 1, 100000, 2, 32, 64, 1600000
NCORES = 8
BLK = 128           # edges per tile (matmul contraction)
BLK_V = 64          # vertices per block / one-hot width
EW = 2 * F + 4      # bf16 words per edge row: 64 hi | 64 lo | negv f32 (2) | pad (2)
ADJ_G = 7

ONE_F32_U16 = np.array([0x0000, 0x3F80], dtype=np.uint16)  # f32 1.0 as 2 LE u16


class Cfg:
    def __init__(self, v=V, ncores=NCORES):
        self.V = v
        self.NCORES = ncores
        self.VLOC = v // ncores
        self.NBLK = (self.VLOC + BLK_V - 1) // BLK_V
        self.VPAD = self.NBLK * BLK_V


_DEFAULT_CFG = Cfg()
_NC_CACHE = {}


def build_device_program(profile, cfg=_DEFAULT_CFG, act_frac=0.22):
    """Build + compile the per-core Bass program.

    profile: per-slot tile counts (len NBLK); same static schedule on all cores.
    act_frac: fraction of one-hot builds routed to ScalarE (2-op trick) to
    offload the Vector engine.
    """
    from contextlib import ExitStack

    import concourse.tile as tile
    from concourse import bacc, mybir

    f32 = mybir.dt.float32
    bf16 = mybir.dt.bfloat16
    i32 = mybir.dt.int32
    NBLK = cfg.NBLK
    assert len(profile) == NBLK and NBLK % ADJ_G == 0
    t_max = max(profile)
    slot_elems = [BLK * ts * EW for ts in profile]  # edge rows are 128/tile
    slot_base = np.concatenate([[0], np.cumsum(slot_elems)]).astype(np.int64)
    total_elems = int(slot_base[-1])

    nc = bacc.Bacc("TRN2", target_bir_lowering=False, debug=False)
    feat_d = [
        nc.dram_tensor(f"feat{t}", [total_elems], bf16, kind="ExternalInput").ap()
        for t in range(T)
    ]
    adj_d = nc.dram_tensor(
        "adj", [NBLK // ADJ_G, BLK_V, ADJ_G * T * N], i32, kind="ExternalInput"
    ).ap()
    # iota_neg[e, j] = -j (f32) for DVE is_equal against negv;
    # iota_pos[e, j] = +j (bf16) for the ScalarE (j + negv)^2 path
    iota_n_d = nc.dram_tensor("iota_neg", [BLK, BLK_V], f32, kind="ExternalInput").ap()
    iota_p_d = nc.dram_tensor("iota_pos", [BLK, BLK_V], bf16, kind="ExternalInput").ap()
    out_d = nc.dram_tensor("out", [NBLK, BLK_V, T * F], f32, kind="ExternalOutput").ap()

    with tile.TileContext(nc) as tc, ExitStack() as ctx:
        const = ctx.enter_context(tc.tile_pool(name="const", bufs=1))
        featp = ctx.enter_context(tc.tile_pool(name="featp", bufs=4))
        adjp = ctx.enter_context(tc.tile_pool(name="adjp", bufs=2))
        degp = ctx.enter_context(tc.tile_pool(name="degp", bufs=3))
        ohp = ctx.enter_context(tc.tile_pool(name="ohp", bufs=8))
        redp = ctx.enter_context(tc.tile_pool(name="redp", bufs=3))
        outp = ctx.enter_context(tc.tile_pool(name="outp", bufs=3))
        psump = ctx.enter_context(tc.tile_pool(name="psum", bufs=4, space="PSUM"))

        iota_n = const.tile([BLK, BLK_V], f32)
        nc.sync.dma_start(out=iota_n[:], in_=iota_n_d[:])
        iota_p = const.tile([BLK, BLK_V], bf16)
        nc.sync.dma_start(out=iota_p[:], in_=iota_p_d[:])

        oh_seq = 0

        def build_onehot(oh, negv_ap):
            nonlocal oh_seq
            use_act = int((oh_seq + 1) * act_frac) > int(oh_seq * act_frac)
            oh_seq += 1
            if use_act:
                y = ohp.tile([BLK, BLK_V], bf16, tag="y")
                nc.scalar.activation(
                    y[:], iota_p[:], mybir.ActivationFunctionType.Square,
                    bias=negv_ap, scale=1.0,
                )
                nc.scalar.activation(
                    oh[:], y[:], mybir.ActivationFunctionType.Relu,
                    bias=1.0, scale=-1.0,
                )
            else:
                nc.vector.tensor_scalar(
                    oh[:], iota_n[:], negv_ap, None, op0=mybir.AluOpType.is_equal
                )

        for bg in range(NBLK // ADJ_G):
            adj_t = adjp.tile([BLK_V, ADJ_G * T * N], i32)
            nc.sync.dma_start(out=adj_t[:], in_=adj_d[bg])
            val = degp.tile([BLK_V, ADJ_G * T * N], f32, tag="val")
            nc.vector.tensor_scalar(
                val[:], adj_t[:], 0, None, op0=mybir.AluOpType.is_ge
            )
            deg = degp.tile([BLK_V, ADJ_G * T], f32, tag="deg")
            nc.vector.tensor_reduce(
                deg[:],
                val[:].rearrange("p (g n) -> p g n", n=N),
                axis=mybir.AxisListType.X,
                op=mybir.AluOpType.add,
            )
            rec = degp.tile([BLK_V, ADJ_G * T], f32, tag="rec")
            nc.vector.tensor_scalar(
                deg[:], deg[:], 1.0, None, op0=mybir.AluOpType.max
            )
            nc.vector.reciprocal(rec[:], deg[:])

            for bo in range(ADJ_G):
                s = bg * ADJ_G + bo
                t_s = profile[s]
                out_t = outp.tile([BLK_V, T * F], f32)
                for t in range(T):
                    feat_t = featp.tile([BLK, t_max * EW], bf16, tag="feat")
                    src = feat_d[t][
                        int(slot_base[s]) : int(slot_base[s + 1])
                    ].rearrange("(e w) -> e w", w=t_s * EW)
                    nc.sync.dma_start(out=feat_t[:, : t_s * EW], in_=src)
                    ps = psump.tile([BLK_V, 2 * F], f32)
                    for i in range(t_s):
                        oh = ohp.tile([BLK, BLK_V], bf16, tag="oh")
                        negv = feat_t[:, i * EW + 2 * F : i * EW + 2 * F + 2].bitcast(f32)
                        build_onehot(oh, negv)
                        nc.tensor.matmul(
                            ps[:],
                            lhsT=oh[:],
                            rhs=feat_t[:, i * EW : i * EW + 2 * F],
                            start=(i == 0),
                            stop=(i == t_s - 1),
                        )
                    # sum hi+lo halves: [128, (2,64)] -> [128, 64]
                    red = redp.tile([BLK_V, F], f32)
                    nc.vector.tensor_reduce(
                        red[:],
                        ps[:].rearrange("p (h f) -> p f h", h=2),
                        axis=mybir.AxisListType.X,
                        op=mybir.AluOpType.add,
                    )
                    # mean = sum * (1/deg) on ScalarE
                    nc.scalar.mul(
                        out_t[:, t * F : (t + 1) * F],
                        red[:],
                        rec[:, bo * T + t : bo * T + t + 1],
                    )
                nc.sync.dma_start(out=out_d[s], in_=out_t[:])

    nc.compile()
    return nc


def shard_table(indices, cfg=_DEFAULT_CFG):
    """Sort edges by destination and partition by core.

    Returns per-core list of (orig_edge_idx sorted by dest, block, rank_in_block,
    tiles_per_block)."""
    v = np.ascontiguousarray(indices[:, 1])
    order = np.argsort(v, kind="stable")
    vs = v[order]
    bounds = np.searchsorted(vs, np.arange(cfg.NCORES + 1) * cfg.VLOC)
    per_core = []
    for c in range(cfg.NCORES):
        lo, hi = bounds[c], bounds[c + 1]
        idx = order[lo:hi]
        vloc = vs[lo:hi].astype(np.int64) - c * cfg.VLOC
        blk = vloc // BLK_V
        vin = vloc % BLK_V
        cnt = np.bincount(blk, minlength=cfg.NBLK).astype(np.int64)
        starts = np.zeros(cfg.NBLK, dtype=np.int64)
        np.cumsum(cnt[:-1], out=starts[1:])
        rank = np.arange(len(idx), dtype=np.int64) - starts[blk]
        tiles = (cnt + BLK - 1) // BLK
        per_core.append((idx, blk, vin, rank, tiles))
    return per_core


def make_profile(per_core_tables, cfg=_DEFAULT_CFG):
    """Slot tile profile + per (core, table) block->slot permutation."""
    perms = []  # perms[t][c] = array: slot -> block
    sorted_tiles = []
    for per_core in per_core_tables:
        perms_t = []
        for c in range(cfg.NCORES):
            tiles = per_core[c][4]
            order = np.argsort(-tiles, kind="stable")
            perms_t.append(order)
            sorted_tiles.append(tiles[order])
        perms.append(perms_t)
    profile = np.max(np.stack(sorted_tiles), axis=0)
    profile = np.maximum(profile, 1)
    return [int(x) for x in profile], perms


def fill_feature_stream(per_core, features, profile, perm_t, cfg=_DEFAULT_CFG):
    """Per-core bf16 edge stream, slot-major, edge-slot-major within a slot.

    Row layout (130 bf16 words): [hi(64) | lo(64) | negv as f32 (2 words)].
    Padding rows have negv = +1.0 (never matches iota_neg <= 0)."""
    prof = np.asarray(profile, dtype=np.int64)
    row_base = np.concatenate([[0], np.cumsum(prof * BLK)]).astype(np.int64)
    total_rows = int(row_base[-1])

    hi = features.astype(ml_dtypes.bfloat16)
    lo = (features - hi.astype(np.float32)).astype(ml_dtypes.bfloat16)
    hi_u = hi.view(np.uint16)
    lo_u = lo.view(np.uint16)

    out = np.zeros((cfg.NCORES, total_rows, EW), dtype=np.uint16)
    out[:, :, 2 * F : 2 * F + 2] = ONE_F32_U16  # negv = +1.0 for padding rows
    for c in range(cfg.NCORES):
        idx, blk, vin, rank, _tiles = per_core[c]
        inv = np.empty(cfg.NBLK, dtype=np.int64)
        inv[perm_t[c]] = np.arange(cfg.NBLK)
        s = inv[blk]
        rows = row_base[s] + (rank & 127) * prof[s] + (rank >> 7)
        out[c, rows, 0:F] = hi_u[idx]
        out[c, rows, F : 2 * F] = lo_u[idx]
        out[c, rows, 2 * F : 2 * F + 2] = (
            (-vin.astype(np.float32)).view(np.uint32).view(np.uint16).reshape(-1, 2)
        )
    return out.reshape(cfg.NCORES, total_rows * EW).view(ml_dtypes.bfloat16)


def prep_adjacency(adjacency, perms, cfg=_DEFAULT_CFG):
    """adj_dev[c, g, vin, j*64 + t*32 + n] = adjacency[0, block_{t}(c, 7g+j), vin, t, n]
    padded with -1 beyond VLOC."""
    adj = np.ascontiguousarray(adjacency.reshape(cfg.V, T, N))
    adj_pad = np.full((cfg.NCORES, cfg.VPAD, T, N), -1, dtype=np.int32)
    adj_pad[:, : cfg.VLOC] = adj.reshape(cfg.NCORES, cfg.VLOC, T, N)
    adj_pad = adj_pad.reshape(cfg.NCORES, cfg.NBLK, BLK_V, T, N)
    out = np.empty((cfg.NCORES, cfg.NBLK, BLK_V, T, N), dtype=np.int32)
    for c in range(cfg.NCORES):
        for t in range(T):
            out[c, :, :, t, :] = adj_pad[c, perms[t][c], :, t, :]
    # [c, g, j, vin, t, n] -> [c, g, vin, j, t, n]
    out = out.reshape(cfg.NCORES, cfg.NBLK // ADJ_G, ADJ_G, BLK_V, T * N)
    out = np.ascontiguousarray(out.transpose(0, 1, 3, 2, 4))
    return out.reshape(cfg.NCORES, cfg.NBLK // ADJ_G, BLK_V, ADJ_G * T * N)


def prepare_inputs(adjacency, indices0, features0, indices1, features1, cfg=_DEFAULT_CFG):
    adjacency = np.asarray(adjacency)
    pc0 = shard_table(np.asarray(indices0), cfg)
    pc1 = shard_table(np.asarray(indices1), cfg)
    profile, perms = make_profile([pc0, pc1], cfg)

    f0 = fill_feature_stream(
        pc0, np.asarray(features0, dtype=np.float32), profile, perms[0], cfg
    )
    f1 = fill_feature_stream(
        pc1, np.asarray(features1, dtype=np.float32), profile, perms[1], cfg
    )
    adj = prep_adjacency(adjacency, perms, cfg)
    iota_neg = np.broadcast_to(
        -np.arange(BLK_V, dtype=np.float32), (BLK, BLK_V)
    ).copy()
    iota_pos = np.broadcast_to(
        np.arange(BLK_V).astype(ml_dtypes.bfloat16), (BLK, BLK_V)
    ).copy()

    in_maps = [
        {
            "feat0": f0[c],
            "feat1": f1[c],
            "adj": adj[c],
            "iota_neg": iota_neg,
            "iota_pos": iota_pos,
        }
        for c in range(cfg.NCORES)
    ]
    return in_maps, profile, perms


def assemble_output(core_outs, perms, cfg=_DEFAULT_CFG):
    outs = []
    for t in range(T):
        parts = []
        for c in range(cfg.NCORES):
            res_t = core_outs[c].reshape(cfg.NBLK, BLK_V, T, F)[:, :, t, :]
            tmp = np.empty((cfg.NBLK, BLK_V, F), dtype=res_t.dtype)
            tmp[perms[t][c]] = res_t
            parts.append(tmp.reshape(cfg.VPAD, F)[: cfg.VLOC])
        outs.append(np.concatenate(parts, axis=0).reshape(B, cfg.V, F))
    return (outs[0], outs[1])


def kernel(adjacency, indices0, features0, indices1, features1):
    from concourse.bass_utils import run_bass_kernel_spmd

    cfg = _DEFAULT_CFG
    in_maps, profile, perms = prepare_inputs(
        adjacency, indices0, features0, indices1, features1, cfg
    )

    key = tuple(profile)
    if key not in _NC_CACHE:
        _NC_CACHE[key] = build_device_program(profile, cfg)
    nc = _NC_CACHE[key]

    res = run_bass_kernel_spmd(nc, in_maps, list(range(cfg.NCORES)))
    return assemble_output(
        [res.results[c]["out"] for c in range(cfg.NCORES)], perms, cfg
    )
